# revision 56
# baseline (speedup 1.0000x reference)
"""Causal self-attention (B=2, S=2048, D=1024, H=16, Dh=64) on 8 NeuronCores.

Sharding: core c -> batch b = c//4, head-group g = c%4 (heads 4g..4g+3).
Each core computes QKV projection for its 4 heads, causal attention
(scores kept transposed: [k, q] layout), and a partial output projection
over its local head dims. Host sums the 4 partials per batch, adds b_out.

v8 line: restructured pipeline (169us vs the 228us v7 baseline).
Attention runs per 512-query chunk: for each (head-pair hp, query-chunk
jq) the key tiles stream through a 2-bank fp32 score tile (both heads
side by side, one exp per unit, two in flight), accumulating into a
single 2-bank [65,1024] attn PSUM tile (col 64 of the augmented V gives
the softmax denominator as row 64). attn@V is deferred TWO slots so the
per-slot gate is the 2-slot-old exp, matching the score-bank rotation.
PSUM pools: scores 2x2 banks, attn 2, projections 2 — projection /
out-projection / normalize traffic never shares slots with the score
pipeline. The causal diagonal spans the last 4 key tiles of each chunk
(off=128t): scores/exp/attnv narrow accordingly; gpsimd affine_select
masks the one triangle block. Normalization: copy-evacuate to SBUF,
ones-column broadcast matmul + fast reciprocal, rescale during the
out-projection path. QKV / V / out-projection groups are hand-placed
fillers inside attention slots. Inputs are host-rearranged to SBUF
layouts so DMAs are few large contiguous transfers, striped over the
sync/scalar/gpsimd rings in compute-need order (each dma_start costs
~0.6us on its queue; the scalar ring's triggers finish before the exp
stream begins). Output is ec-major for contiguous DRAM writes.
"""

import numpy as np
from contextlib import ExitStack

B = 2
S = 2048
D = 1024
NH = 16
DH = 64
N_CORES = 8
HPC = 4            # heads per core
EL = HPC * DH      # 256 local head dims per core
KD = D // 128      # 8 contraction chunks for projections
KT = S // 128      # 16 key tiles

_NC = None
_last_in_maps = None


def _build_program():
    import concourse.mybir as mybir
    import concourse.tile as tile
    from concourse import bacc

    F32 = mybir.dt.float32
    BF16 = mybir.dt.bfloat16
    Exp = mybir.ActivationFunctionType.Exp

    nc = bacc.Bacc("TRN2", target_bir_lowering=False, debug=False,
                   num_devices=N_CORES)

    # dram layouts mirror the SBUF tiles exactly (host pre-arranges):
    # xt cols (sc, k, c); wqa/wqb cols (k, qk*128+c); wv cols (k, 260)
    xt_d = nc.dram_tensor("xt", [128, 4 * KD * 512], BF16,
                          kind="ExternalInput")
    wqa_d = nc.dram_tensor("wqa", [128, KD * 256], BF16,
                           kind="ExternalInput")
    wqb_d = nc.dram_tensor("wqb", [128, KD * 256], BF16,
                           kind="ExternalInput")
    wv_d = nc.dram_tensor("wv", [128, KD * 260], BF16,
                          kind="ExternalInput")
    bqk_d = nc.dram_tensor("bqk", [128, 4], F32, kind="ExternalInput")
    ones_d = nc.dram_tensor("ones", [1, 512], BF16, kind="ExternalInput")
    bv_d = nc.dram_tensor("bv", [1, HPC * 65], BF16, kind="ExternalInput")
    wo_d = nc.dram_tensor("wo", [EL, D], BF16, kind="ExternalInput")
    # ec-major output: rows [2048*ec + s] — each [128,512] store is a
    # fully contiguous 128KB DRAM write (strided writes halve ring BW)
    out_d = nc.dram_tensor("out", [2 * S, 512], BF16, kind="ExternalOutput")

    with nc.allow_low_precision(reason="bf16 matmul operands, fp32 accum"), \
         tile.TileContext(nc) as tc, ExitStack() as ctx:
        const = ctx.enter_context(tc.tile_pool(name="const", bufs=1))
        pin = ctx.enter_context(tc.tile_pool(name="pin", bufs=1))
        work = ctx.enter_context(tc.tile_pool(name="work", bufs=1))
        psS = ctx.enter_context(tc.tile_pool(name="psS", bufs=2, space="PSUM"))
        psOT = ctx.enter_context(tc.tile_pool(name="psOT", bufs=1, space="PSUM"))
        psPJ = ctx.enter_context(tc.tile_pool(name="psPJ", bufs=2, space="PSUM"))
        pp = ctx.enter_context(tc.tile_pool(name="pp", bufs=3))
        sm = ctx.enter_context(tc.tile_pool(name="sm", bufs=2))
        ob = ctx.enter_context(tc.tile_pool(name="ob", bufs=4))

        # ---------------- input tiles (consolidated, host-prearranged) ---
        # xt_all cols: (sc, k, c) — sc-major so DMA order = need order,
        # k-contiguous 512-col slices for QK rhs, 128-col for V lhsT.
        xt_all = pin.tile([128, 4 * KD * 512], BF16, tag="xt", name="xt")
        # wqa/wqb cols: (k, qk*128+c); wv cols: (k, 260)
        wqa_all = pin.tile([128, KD * 256], BF16, tag="wqa", name="wqa")
        wqb_all = pin.tile([128, KD * 256], BF16, tag="wqb", name="wqb")
        wv_all = pin.tile([128, KD * 260], BF16, tag="wv", name="wv")
        wo_s = [const.tile([128, D], BF16, tag=f"wo{i}", name=f"wo{i}")
                for i in range(2)]
        bqk_s = const.tile([128, 4], F32, tag="bqk", name="bqk")
        bv_s = const.tile([1, HPC * 65], BF16, tag="bv", name="bv")
        ones_s = const.tile([1, 512], BF16, tag="ones", name="ones")
        ones64_s = const.tile([65, 64], BF16, tag="ones64", name="ones64")

        def XT(k, sc):          # [128, 512] rhs slice for QK proj
            c0 = 4096 * sc + 512 * k
            return xt_all[:, c0:c0 + 512]

        def XTst(k, st):        # [128, 128] lhsT slice for V proj
            c0 = 4096 * (st // 4) + 512 * k + 128 * (st % 4)
            return xt_all[:, c0:c0 + 128]

        def WQ(hp, k, qk):      # [128, 128] lhsT slice for Q/K proj
            t = wqa_all if hp == 0 else wqb_all
            c0 = 256 * k + 128 * qk
            return t[:, c0:c0 + 128]

        def WV(k):              # [128, 260] rhs slice for V proj
            return wv_all[:, 260 * k:260 * (k + 1)]

        # ---------------- input DMAs: few large transfers, need order ---
        # Only gpsimd/sync/scalar queues may start DMAs. Each trigger
        # costs ~0.6us on its queue and transfers serialize per ring, so
        # stripe by need time; the scalar queue's triggers all finish
        # before the exp stream begins.
        hv = KD * 260 // 2
        # first 1.5MB split into small pieces across 3 rings so phase A
        # can start ~8us and is paced by 2 rings, not one
        nc.sync.dma_start(out=wqa_all[:, 0:1024], in_=wqa_d[:, 0:1024])
        nc.scalar.dma_start(out=xt_all[:, 0:1024], in_=xt_d[:, 0:1024])
        nc.gpsimd.dma_start(out=xt_all[:, 1024:2048], in_=xt_d[:, 1024:2048])
        nc.sync.dma_start(out=wqa_all[:, 1024:2048], in_=wqa_d[:, 1024:2048])
        nc.scalar.dma_start(out=xt_all[:, 2048:3072], in_=xt_d[:, 2048:3072])
        nc.gpsimd.dma_start(out=bqk_s, in_=bqk_d[:, :])
        nc.gpsimd.dma_start(out=bv_s, in_=bv_d[:, :])
        nc.gpsimd.dma_start(out=ones_s, in_=ones_d[:, :])
        nc.gpsimd.dma_start(out=ones64_s[64:65, :], in_=ones_d[0:1, 0:64])
        nc.scalar.dma_start(out=xt_all[:, 3072:4096], in_=xt_d[:, 3072:4096])
        nc.sync.dma_start(out=wv_all[:, 0:hv], in_=wv_d[:, 0:hv])
        nc.gpsimd.dma_start(out=wv_all[:, hv:], in_=wv_d[:, hv:])
        nc.gpsimd.dma_start(out=wqb_all, in_=wqb_d[:, :])
        nc.sync.dma_start(out=xt_all[:, 4096:6144], in_=xt_d[:, 4096:6144])
        nc.sync.dma_start(out=xt_all[:, 6144:8192], in_=xt_d[:, 6144:8192])
        nc.scalar.dma_start(out=xt_all[:, 8192:10240], in_=xt_d[:, 8192:10240])
        nc.scalar.dma_start(out=xt_all[:, 10240:12288], in_=xt_d[:, 10240:12288])
        nc.sync.dma_start(out=xt_all[:, 12288:14336], in_=xt_d[:, 12288:14336])
        nc.scalar.dma_start(out=xt_all[:, 14336:16384], in_=xt_d[:, 14336:16384])
        nc.sync.dma_start(out=wo_s[0], in_=wo_d[0:128, :])
        nc.scalar.dma_start(out=wo_s[1], in_=wo_d[128:256, :])

        # ---------------- persistent SBUF tensors ----------------
        # qk_s: 0=Q hp0, 1=K hp0, 2=Q hp1, 3=K hp1; [dim(2 heads), seq]
        qk_s = [work.tile([128, S], BF16, tag=f"qk{e}", name=f"qk{e}")
                for e in range(4)]
        # V augmented: per key-tile [128, 4*65]; col 64 of each head = 1.0
        vaug_s = [work.tile([128, HPC * 65], BF16, tag=f"va{t}", name=f"va{t}")
                  for t in range(KT)]
        # normalized attn output, transposed: [d_local, s]
        attnT_s = [work.tile([128, S], BF16, tag=f"at{d}", name=f"at{d}")
                   for d in range(2)]

        # warm the ACT exp table off the critical path
        dummy = sm.tile([1, 16], BF16, tag="dummy", name="dummy")
        nc.scalar.activation(out=dummy, in_=ones_s[0:1, 0:16], func=Exp,
                             scale=1.0)
        # zero weights for keep-warm filler matmuls in sparse slots
        zeros_s = const.tile([128, 65], BF16, tag="zeros", name="zeros")
        nc.gpsimd.memset(zeros_s[:, :], 0.0)

        # ---------------- emission helpers ----------------
        def emit_qk_group(hp, qk, sc):
            """One [128,512] Q-or-K projection group. qk: 0=Q, 1=K."""
            e = 2 * hp + qk
            ps = psPJ.tile([128, 512], F32, tag="pj", name=f"qg{e}{sc}")
            for k in range(KD):
                nc.tensor.matmul(
                    ps, lhsT=WQ(hp, k, qk), rhs=XT(k, sc),
                    start=(k == 0), stop=(k == KD - 1))
            nc.vector.tensor_scalar_add(
                out=qk_s[e][:, 512 * sc:512 * (sc + 1)], in0=ps,
                scalar1=bqk_s[:, e:e + 1])

        def emit_qk2_interleaved(hp, sc):
            """Both Q and K groups of a pair, k-interleaved (DMA pacing)."""
            psq = psPJ.tile([128, 512], F32, tag="pj", name=f"qgA{hp}{sc}")
            psk = psPJ.tile([128, 512], F32, tag="pj", name=f"qgB{hp}{sc}")
            for k in range(KD):
                nc.tensor.matmul(
                    psk, lhsT=WQ(hp, k, 1), rhs=XT(k, sc),
                    start=(k == 0), stop=(k == KD - 1))
                nc.tensor.matmul(
                    psq, lhsT=WQ(hp, k, 0), rhs=XT(k, sc),
                    start=(k == 0), stop=(k == KD - 1))
            eq, ek = 2 * hp, 2 * hp + 1
            nc.vector.tensor_scalar_add(
                out=qk_s[ek][:, 512 * sc:512 * (sc + 1)], in0=psk,
                scalar1=bqk_s[:, ek:ek + 1])
            nc.vector.tensor_scalar_add(
                out=qk_s[eq][:, 512 * sc:512 * (sc + 1)], in0=psq,
                scalar1=bqk_s[:, eq:eq + 1])

        # V-bias as a pre-broadcast SBUF tile, added during evacuation
        # (saves a tiny K=1 matmul per V group)
        bvb_s = const.tile([128, HPC * 65], BF16, tag="bvb", name="bvb")

        def emit_vbias_bcast():
            ps = psPJ.tile([128, HPC * 65], F32, tag="pj", name="bvb_ps")
            nc.tensor.matmul(ps, lhsT=ones_s[0:1, 0:128], rhs=bv_s,
                             start=True, stop=True)
            nc.vector.tensor_copy(bvb_s, ps)

        def emit_v_group(st):
            ps = psPJ.tile([128, HPC * 65], F32, tag="pj", name=f"vg{st}")
            for k in range(KD):
                nc.tensor.matmul(
                    ps, lhsT=XTst(k, st), rhs=WV(k),
                    start=(k == 0), stop=(k == KD - 1))
            nc.vector.tensor_add(out=vaug_s[st], in0=ps, in1=bvb_s)

        def emit_c_group(st, ec, tail=False):
            ps = psPJ.tile([128, 512], F32, tag="pj", name=f"cg{st}{ec}")
            for dl in range(2):
                nc.tensor.matmul(
                    ps,
                    lhsT=attnT_s[dl][:, 128 * st:128 * (st + 1)],
                    rhs=wo_s[dl][:, 512 * ec:512 * (ec + 1)],
                    start=(dl == 0), stop=(dl == 1))
            o_t = ob.tile([128, 512], BF16, tag="ob", name="ob")
            dst = out_d[2048 * ec + 128 * st:2048 * ec + 128 * (st + 1), :]
            if not tail:
                # in-loop: keep the scalar queue clean for the exp stream
                nc.vector.tensor_copy(o_t, ps)
                nc.sync.dma_start(out=dst, in_=o_t)
            elif ec == 0:
                nc.vector.tensor_copy(o_t, ps)
                nc.gpsimd.dma_start(out=dst, in_=o_t)
            else:
                nc.scalar.copy(out=o_t, in_=ps)
                nc.scalar.dma_start(out=dst, in_=o_t)

        # ---------------- attention ----------------
        # Unit geometry: chunk (hp, jq) covers queries [512jq, 512jq+512),
        # key tiles ki = 0..4jq+3. The last four key tiles (t = ki-4jq in
        # 0..3) straddle the causal diagonal: query cols < 128t are fully
        # masked (skipped), cols [128t, 128t+128) are the triangle block
        # (gpsimd affine_select after exp), the rest is fully valid.
        def unit_off(jq, ki):
            t = ki - 4 * jq
            return 128 * t if t >= 0 else -1   # -1: interior (no mask)

        def emit_scores(hp, jq, ki):
            off = max(0, unit_off(jq, ki))
            qQ = qk_s[2 * hp]
            qK = qk_s[2 * hp + 1]
            q0 = 512 * jq
            s_t = psS.tile([128, 1024], F32, tag="s", name=f"s{hp}{jq}{ki}")
            for h2 in range(2):
                nc.tensor.matmul(
                    s_t[:, 512 * h2 + off:512 * (h2 + 1)],
                    lhsT=qK[64 * h2:64 * h2 + 64, 128 * ki:128 * (ki + 1)],
                    rhs=qQ[64 * h2:64 * h2 + 64, q0 + off:q0 + 512],
                    start=True, stop=True)
            return s_t

        def emit_exp(s_t, jq, ki):
            off = unit_off(jq, ki)
            p_t = pp.tile([128, 1024], BF16, tag="p", name="pt")
            if off < 0 or off <= 128:
                # interior, or edge with <=1 dead 128-col block per head:
                # one full-width exp beats two narrow ones (352cyc/instr);
                # dead cols hold stale-score exps that attnv never reads.
                nc.scalar.activation(out=p_t, in_=s_t, func=Exp, scale=0.125)
            else:
                for h2 in range(2):
                    nc.scalar.activation(
                        out=p_t[:, 512 * h2 + off:512 * (h2 + 1)],
                        in_=s_t[:, 512 * h2 + off:512 * (h2 + 1)],
                        func=Exp, scale=0.125)
            if off >= 0:
                for h2 in range(2):
                    nc.gpsimd.affine_select(
                        out=p_t[:, 512 * h2 + off:512 * h2 + off + 128],
                        in_=p_t[:, 512 * h2 + off:512 * h2 + off + 128],
                        compare_op=mybir.AluOpType.is_ge, fill=0.0,
                        base=0, pattern=[[1, 128]], channel_multiplier=-1)
            return p_t

        def emit_attnv(hp, jq, ki, p_t, ot_t):
            off = max(0, unit_off(jq, ki))
            for h2 in range(2):
                h = 2 * hp + h2
                nc.tensor.matmul(
                    ot_t[0:65, 512 * h2 + off:512 * (h2 + 1)],
                    lhsT=vaug_s[ki][:, 65 * h:65 * h + 65],
                    rhs=p_t[:, 512 * h2 + off:512 * (h2 + 1)],
                    start=(ki == 0), stop=(ki == 4 * jq + 3),
                    skip_group_check=True)

        def emit_chunk(hp, jq, sched, carry_in=(), kw=()):
            """One (head-pair, query-chunk): stream key tiles 0..4jq+3.
            attn@V is deferred TWO slots (the per-slot gate becomes the
            2-slot-old exp, matching the score-bank rotation); the last
            two attn@Vs are returned as carries for the next chunk's
            slots 0 and 1. kw: slots (>=3) that get zero-weight filler
            matmuls into the live accumulators to keep the PE clock warm."""
            nki = 4 * jq + 4
            ot_t = psOT.tile([65, 1024], F32, tag="ot", name=f"ot{hp}{jq}")
            pend = []
            for ki in range(nki):
                s_t = emit_scores(hp, jq, ki)
                if ki < len(carry_in):
                    carry_in[ki]()
                if len(pend) == 2:
                    k0, p0 = pend.pop(0)
                    emit_attnv(hp, jq, k0, p0, ot_t)
                p_t = emit_exp(s_t, jq, ki)
                for f in sched.get(ki, ()):
                    f()
                if ki in kw:
                    for h2 in range(2):
                        nc.tensor.matmul(
                            ot_t[0:65, 512 * h2:512 * (h2 + 1)],
                            lhsT=zeros_s[:, 0:65],
                            rhs=qk_s[0][:, 0:512],
                            start=False, stop=False, skip_group_check=True)
                pend.append((ki, p_t))
            carries = [
                (lambda k0=k, p0=p: emit_attnv(hp, jq, k0, p0, ot_t))
                for k, p in pend]
            return carries, ot_t

        def norm_parts(hp, jq, ot_t):
            """Three normalize stages for filler slots 0/1/2 of the next
            chunk: evacuate+den, broadcast+recip, muls+shift."""
            den_b = sm.tile([65, 1024], BF16, tag="den", name="den")
            u_t = sm.tile([65, 1024], BF16, tag="ut", name="ut")
            rb_sb = [sm.tile([64, 512], F32, tag=f"rb{h2}", name=f"rb{h2}")
                     for h2 in range(2)]
            rb_ps = [None, None]

            def n1():
                nc.vector.tensor_copy(den_b[64:65, :], ot_t[64:65, :])
                nc.vector.tensor_copy(u_t[0:64, :], ot_t[0:64, :])

            def n2():
                for h2 in range(2):
                    rp = psPJ.tile([64, 512], F32, tag="pj", name=f"rb{h2}")
                    nc.tensor.matmul(
                        rp, lhsT=ones64_s[64:65, 0:64],
                        rhs=den_b[64:65, 512 * h2:512 * (h2 + 1)],
                        start=True, stop=True)
                    rb_ps[h2] = rp
                for h2 in range(2):
                    nc.vector.reciprocal_approx_fast(
                        out=rb_sb[h2], in_=rb_ps[h2])

            def n3():
                a0 = 512 * jq
                nc.vector.tensor_mul(
                    out=attnT_s[hp][0:64, a0:a0 + 512],
                    in0=u_t[0:64, 0:512], in1=rb_sb[0])
                t_n = sm.tile([64, 512], BF16, tag="tn", name="tn")
                nc.vector.tensor_mul(
                    out=t_n, in0=u_t[0:64, 512:1024], in1=rb_sb[1])
                nc.sync.dma_start(
                    out=attnT_s[hp][64:128, a0:a0 + 512], in_=t_n)

            return n1, n2, n3

        # ---------------- schedule ----------------
        # phase A: first projections, k-interleaved to pace on DMA
        emit_qk2_interleaved(0, 0)
        emit_vbias_bcast()
        emit_v_group(0)

        CHUNKS = [(0, 0), (1, 0), (0, 1), (1, 1), (0, 2), (1, 2),
                  (0, 3), (1, 3)]
        QK = emit_qk_group
        V = emit_v_group
        C = emit_c_group
        cg = {jq: [(st, ec) for st in range(4 * jq, 4 * jq + 4)
                   for ec in range(2)] for jq in range(4)}

        sched = {
            0: {0: [lambda: V(1)],
                1: [lambda: V(2)],
                2: [lambda: V(3), lambda: QK(1, 1, 0)],
                3: [lambda: QK(1, 0, 0)]},
            1: {0: [lambda: QK(0, 0, 1)],
                1: [lambda: QK(0, 1, 1)],
                2: [lambda: V(4)],
                3: [lambda: V(5)]},
            2: {0: [lambda: V(6)],
                1: [lambda: V(7)],
                2: [lambda: QK(1, 0, 1)],
                3: [lambda: QK(1, 1, 1)],
                5: [lambda: C(*cg[0][0])],
                6: [lambda: C(*cg[0][1])],
                7: [lambda: C(*cg[0][2])]},
            3: {1: [lambda: V(8)],
                2: [lambda: V(9)],
                3: [lambda: QK(0, 0, 2)],
                4: [lambda: QK(0, 1, 2)],
                5: [lambda: QK(1, 0, 2)],
                6: [lambda: QK(1, 1, 2)]},
            4: {0: [lambda: V(10)],
                1: [lambda: V(11)],
                5: [lambda: QK(0, 0, 3), lambda: C(*cg[1][0])],
                7: [lambda: QK(0, 1, 3)],
                8: [lambda: C(*cg[1][1])],
                9: [lambda: V(12)],
                11: [lambda: V(13)]},
            5: {4: [lambda: C(*cg[0][6])],
                5: [lambda: C(*cg[0][7])],
                6: [lambda: C(*cg[1][2])],
                8: [lambda: C(*cg[1][3])],
                10: [lambda: C(*cg[1][4])]},
            6: {5: [lambda: QK(1, 0, 3), lambda: C(*cg[2][0])],
                7: [lambda: QK(1, 1, 3)],
                9: [lambda: C(*cg[2][1])],
                11: [lambda: V(14)],
                13: [lambda: V(15)]},
            7: {2: [lambda: C(*cg[2][2])],
                4: [lambda: C(*cg[2][3])],
                5: [lambda: C(*cg[1][5])],
                6: [lambda: C(*cg[2][4])],
                8: [lambda: C(*cg[2][5])],
                9: [lambda: C(*cg[1][6])],
                10: [lambda: C(*cg[2][6])],
                11: [lambda: C(*cg[0][3])],
                12: [lambda: C(*cg[0][4])],
                13: [lambda: C(*cg[0][5])]},
        }
        KW = {}

        carry = ()
        prev_norm = None      # (hp, jq, ot_t) of previous chunk
        for ci, (hp, jq) in enumerate(CHUNKS):
            sc = dict(sched[ci])
            if prev_norm is not None:
                n1, n2, n3 = norm_parts(*prev_norm)
                # rb matmul (n2) later on larger chunks so its den
                # dependency is comfortably done when it reaches the
                # PE queue head
                s2, s3 = (2, 3) if jq == 0 else (4, 5)
                sc.setdefault(1, [])
                sc.setdefault(s2, [])
                sc.setdefault(s3, [])
                sc[1] = [n1] + list(sc[1])
                sc[s2] = [n2] + list(sc[s2])
                sc[s3] = [n3] + list(sc[s3])
            carry, ot_t = emit_chunk(hp, jq, sc, carry_in=carry,
                                     kw=KW.get(ci, ()))
            prev_norm = (hp, jq, ot_t)

        # tail: last two attn@Vs, final normalize, final c-chunk
        # (a held-back c(jq2) group keeps the PE busy through the chain)
        for cfn in carry:
            cfn()
        n1, n2, n3 = norm_parts(*prev_norm)
        n1()
        emit_c_group(*cg[2][7], tail=True)
        n2()
        emit_c_group(*cg[1][7], tail=True)
        n3()
        for st, ec in cg[3]:
            emit_c_group(st, ec, tail=True)

    nc.compile()
    return nc


def _get_program():
    global _NC
    if _NC is None:
        _NC = _build_program()
    return _NC


def kernel(x, w_qkv, b_qkv, w_out, b_out):
    import ml_dtypes
    from concourse.bass_utils import run_bass_kernel_spmd

    BF = ml_dtypes.bfloat16
    x = np.asarray(x, dtype=np.float32)
    w_qkv = np.asarray(w_qkv, dtype=np.float32)
    b_qkv = np.asarray(b_qkv, dtype=np.float32)
    w_out = np.asarray(w_out, dtype=np.float32)
    b_out = np.asarray(b_out, dtype=np.float32)

    nc = _get_program()

    in_maps = []
    for c in range(N_CORES):
        b = c // 4
        g = c % 4
        hs = slice(g * EL, (g + 1) * EL)
        wq = w_qkv[0 * D:1 * D][hs]          # [256, 1024]
        wk = w_qkv[1 * D:2 * D][hs]
        wv = w_qkv[2 * D:3 * D][hs]
        bq = b_qkv[0 * D:1 * D][hs]
        bk = b_qkv[1 * D:2 * D][hs]
        bv = b_qkv[2 * D:3 * D][hs]
        # head-pair split: A = heads 0,1 (Q|K), B = heads 2,3
        wqa = np.concatenate([wq[0:128], wk[0:128]])      # [256, 1024]
        wqb = np.concatenate([wq[128:256], wk[128:256]])
        bqk = np.stack([bq[0:128], bk[0:128], bq[128:256], bk[128:256]],
                       axis=1)                            # [128, 4]
        wvx = np.zeros((D, HPC * 65), dtype=np.float32)
        bvx = np.zeros((1, HPC * 65), dtype=np.float32)
        for h in range(HPC):
            wvx[:, 65 * h:65 * h + 64] = wv[h * DH:(h + 1) * DH].T
            bvx[0, 65 * h:65 * h + 64] = bv[h * DH:(h + 1) * DH]
            bvx[0, 65 * h + 64] = 1.0

        # rearrange to the SBUF layouts: xt [128, (sc,k,c)], weights
        # [128, (k,cols)] — so device DMAs are plain contiguous slices.
        xt = np.ascontiguousarray(x[b].T)                 # [1024, 2048]
        xt = xt.reshape(KD, 128, 4, 512).transpose(1, 2, 0, 3)
        xt = xt.reshape(128, 4 * KD * 512)

        def kmajor(w):          # [1024, C] -> [128, KD*C]
            c = w.shape[1]
            return np.ascontiguousarray(
                w.reshape(KD, 128, c).transpose(1, 0, 2).reshape(128, KD * c))

        in_maps.append({
            "xt": xt.astype(BF),                          # [128, 16384]
            "wqa": kmajor(wqa.T.copy()).astype(BF),       # [128, 2048]
            "wqb": kmajor(wqb.T.copy()).astype(BF),
            "wv": kmajor(wvx).astype(BF),                 # [128, 2080]
            "bqk": np.ascontiguousarray(bqk),             # [128, 4] f32
            "bv": bvx.astype(BF),                         # [1, 260]
            "ones": np.ones((1, 512), dtype=BF),
            "wo": np.ascontiguousarray(w_out[:, hs].T).astype(BF),  # [256, 1024]
        })

    global _last_in_maps
    _last_in_maps = in_maps
    res = run_bass_kernel_spmd(nc, in_maps, list(range(N_CORES)))

    out = np.empty((B, S, D), dtype=np.float32)
    for b in range(B):
        acc = res.results[4 * b]["out"].astype(np.float32)
        for j in range(1, 4):
            acc = acc + res.results[4 * b + j]["out"].astype(np.float32)
        # undo the ec-major output layout [2*S, 512] -> [S, 1024]
        out[b] = np.concatenate([acc[0:S], acc[S:2 * S]], axis=1) \
            + b_out[None, :]
    return out
